# revision 6
# baseline (speedup 1.0000x reference)
"""Cross-attention kernel for Trainium2, 8 NeuronCores.

Reference computation (per batch b, with n = h*w = 9216, c = 128, cq = 16):
    q  = (w_q @ y_b)                       # [cq, n]   (used transposed)
    k  = (w_k @ y_b)                       # [cq, n]
    s  = q^T @ k                           # [n, n]    scores
    m  = softmax(s, axis=-1)
    v  = (w_v @ x_b)                       # [c, n]
    out = v @ m^T                          # [c, n]

Sharding: 8 cores = (batch b in {0,1}) x (query block qb in {0..3}, 2304
queries each). Each core sees all 9216 keys.

Pipeline layout (per core): queries in windows of 512 (plus a 256 tail);
keys in 24 groups of 3x128 chunks.  Scores E_T[key, query] keep keys on
partitions so exp'd tiles feed feat/den matmuls as moving operands with no
transposes.  Softmax max-subtraction is skipped (|s| < ~1 for this weight
scale; exp is numerically safe for any plausible input of this
distribution).

Engine budget (cycles/core; clocks: PE 2.4GHz, ACT 1.2GHz, DVE 0.96GHz):
- All PE operands bf16 (1 cycle/row at any width; fp32 is 4, fp32r needs
  width>=256).  Host pre-converts x/y/weights to bf16, halving input DMA.
- score: 3-way tile_position row-packing (K=16 strips at 0/32/64) ->
  72*2304/3 = 55k cycles.  NOT 4-way: with 4 slots a PSUM bank would be
  shared by two concurrently-writing PE tiles (slots must stay 512-f32 =
  one full bank each; 4 such slots x2 bufs leaves no banks for feat/den).
  Sharing a bank between two in-flight packed matmuls crashes the runtime.
- feat: 72*2304 = 166k cycles (the irreducible bf16 core).
- den: NOT a full matmul sweep (another 166k): the DVE (otherwise idle in
  the main loop) accumulates exp tiles in bf16 pairs/quads into 6 partials
  per window; ones-matmuls then contract only the partials -> ~42k cycles.
- exp on ACT writes bf16 (2x ACT throughput vs f32 out; total error ~6e-3
  vs the 2e-2 gate).
- VT prep evacuation on ACT (Copy), K/Q proj evacuation on DVE, balancing
  prep across the two non-PE engines.
- The PE queue is in-order: feat/den-final are emitted LAG groups behind
  the score matmuls (software pipelining) so the PE never waits on the
  exp; the DVE finalize (recip+mul) is lagged further so the DVE never
  waits on the den-final matmuls.
"""

import numpy as np

import concourse.bacc as bacc
import concourse.tile as tile
from concourse import mybir

f32 = mybir.dt.float32
bf16 = mybir.dt.bfloat16

P = 128          # partitions / channels
NK = 9216        # keys (h*w)
NQ = 2304        # queries per core
KC = NK // P     # 72 key chunks of 128
CQ = 16          # query/key projection dim
GS = 3           # key chunks per group (3-way tile_position volley)
NG = KC // GS    # 24 groups per window
# Query windows: four of 512 plus a 256 tail.  512-f32 PSUM slots keep each
# packed matmul in its own PSUM bank (mandatory, see header).
W_SPANS = [(0, 512), (512, 512), (1024, 512), (1536, 512), (2048, 256)]
PR = 4           # groups per den partial (6 partials per window)
LAG = 3          # groups of software-pipelining lag for feat/den
FLAG = 3         # extra groups of lag for the DVE finalize

_CACHE = {}


def _build():
    nc = bacc.Bacc(trn_type="TRN2", target_bir_lowering=False, debug=False)
    y = nc.dram_tensor("y", [P, NK], bf16, kind="ExternalInput")
    yq = nc.dram_tensor("yq", [P, NQ], bf16, kind="ExternalInput")
    x = nc.dram_tensor("x", [P, NK], bf16, kind="ExternalInput")
    # w_q^T / w_k^T replicated into 32-row strips ([wT,0,wT,0,wT,0,wT]) so
    # the score matmuls can run row-packed via tile_position.
    wq = nc.dram_tensor("wq", [P, 112], bf16, kind="ExternalInput")
    wk = nc.dram_tensor("wk", [P, 112], bf16, kind="ExternalInput")
    wv = nc.dram_tensor("wv", [P, P], bf16, kind="ExternalInput")    # w_v^T
    o = nc.dram_tensor("o", [P, NQ], f32, kind="ExternalOutput")

    Exp = mybir.ActivationFunctionType.Exp
    Copy = mybir.ActivationFunctionType.Copy

    with tile.TileContext(nc) as tc:
        with (
            tc.tile_pool(name="const", bufs=1) as const,
            tc.tile_pool(name="big", bufs=1) as big,
            tc.tile_pool(name="xs", bufs=2) as xs,
            tc.tile_pool(name="ps", bufs=2, space="PSUM") as ps,
            tc.tile_pool(name="featp", bufs=1, space="PSUM") as featp,
            tc.tile_pool(name="denp", bufs=1, space="PSUM") as denp,
            tc.tile_pool(name="ep", bufs=6) as ep,
            tc.tile_pool(name="dp", bufs=9) as dp,
            tc.tile_pool(name="op", bufs=2) as op,
            tc.tile_pool(name="small", bufs=2) as small,
        ):
            # ---- constants ----
            wq_sb = const.tile([P, 112], bf16, name="wq_sb")
            nc.sync.dma_start(wq_sb, wq.ap())
            wk_sb = const.tile([P, 112], bf16, name="wk_sb")
            nc.sync.dma_start(wk_sb, wk.ap())
            wv_sb = const.tile([P, P], bf16, name="wv_sb")
            nc.sync.dma_start(wv_sb, wv.ap())
            ones_st = const.tile([P, P], f32, name="ones_st")
            nc.vector.memset(ones_st, 1.0)
            ones_sb = const.tile([P, P], bf16, name="ones_sb")
            nc.vector.tensor_copy(ones_sb, ones_st)

            K_sb = big.tile([112, NK], bf16, name="K_sb")
            Q_sb = big.tile([112, NQ], bf16, name="Q_sb")
            VT = big.tile([P, NK], bf16, name="VT")

            # ---- prep ----
            # yq first (the whole Q projection gates the first score matmul),
            # then y/x chunks interleaved.  All bf16: matmuls at 1 cycle/row.
            def emit_proj(i):
                src = y.ap()[:, i * NQ : (i + 1) * NQ] if i < 4 else yq.ap()
                yst = xs.tile([P, NQ], bf16, tag="yst", name=f"yst{i}")
                nc.sync.dma_start(yst, src)
                wr = wk_sb if i < 4 else wq_sb
                dst = K_sb if i < 4 else Q_sb
                dof = i * NQ if i < 4 else 0
                for t, qs in enumerate(range(0, NQ, 512)):
                    qw = min(512, NQ - qs)
                    kp = ps.tile([112, qw], f32, tag="st", name=f"kp{i}_{t}")
                    nc.tensor.matmul(kp, wr, yst[:, qs : qs + qw], start=True, stop=True)
                    nc.vector.tensor_copy(dst[:, dof + qs : dof + qs + qw], kp)

            def emit_vt(i):
                # vT chunks [128 keys, 128 c] = x_chunk^T @ w_v^T; evacuate
                # four chunks per ACT copy (the DVE is the den-accumulation
                # engine in the main loop; keep prep off it where possible).
                xt = xs.tile([P, NQ], bf16, tag="xt", name=f"xt{i}")
                nc.sync.dma_start(xt, x.ap()[:, i * NQ : (i + 1) * NQ])
                nkc = NQ // P  # 18
                for b0 in range(0, nkc, 4):
                    nb = min(4, nkc - b0)
                    vp = ps.tile([P, nb * P], f32, tag="st", name=f"vp{i}_{b0}")
                    for t in range(b0, b0 + nb):
                        nc.tensor.matmul(
                            vp[:, (t - b0) * P : (t - b0 + 1) * P],
                            xt[:, t * P : (t + 1) * P],
                            wv_sb,
                            start=True,
                            stop=True,
                        )
                    kc0 = i * nkc + b0
                    nc.scalar.activation(VT[:, kc0 * P : (kc0 + nb) * P], vp, Copy)

            emit_proj(4)  # yq -> Q_sb
            for i in range(4):
                emit_proj(i)
                emit_vt(i)

            # ---- main loop, software-pipelined ----
            groups = [(wi, g) for wi in range(len(W_SPANS)) for g in range(NG)]
            feat_tiles = {}
            den_tiles = {}
            et_tiles = {}
            et_stash = {}
            partials = {wi: [] for wi in range(len(W_SPANS))}
            fin_queue = []

            def emit_st(wi, g):
                ws, qwd = W_SPANS[wi]
                st = ps.tile([P, GS, 512], f32, tag="st", name=f"st{wi}_{g}")
                for j in range(GS):
                    kc = GS * g + j
                    nc.tensor.matmul(
                        st[:, j, :qwd],
                        K_sb[32 * j : 32 * j + CQ, kc * P : (kc + 1) * P],
                        Q_sb[32 * j : 32 * j + CQ, ws : ws + qwd],
                        start=True,
                        stop=True,
                        tile_position=(32 * j, 0),
                    )
                et = ep.tile([P, GS, 512], bf16, tag="e", name=f"e{wi}_{g}")
                nc.scalar.activation(et[:, :, :qwd], st[:, :, :qwd], Exp)
                et_tiles[(wi, g)] = et
                # den partial accumulation on DVE: PR groups per partial.
                phase = g % PR
                if phase == 0:
                    et_stash[wi] = et
                elif phase == 1:
                    d = dp.tile([P, GS, 512], bf16, tag="d", name=f"d{wi}_{g}")
                    nc.vector.tensor_add(
                        d[:, :, :qwd], et_stash.pop(wi)[:, :, :qwd], et[:, :, :qwd]
                    )
                    partials[wi].append(d)
                else:
                    d = partials[wi][-1]
                    nc.vector.tensor_add(d[:, :, :qwd], d[:, :, :qwd], et[:, :, :qwd])

            def emit_fd(wi, g):
                ws, qwd = W_SPANS[wi]
                if g == 0:
                    feat_tiles[wi] = featp.tile(
                        [P, qwd], f32, tag="feat", name=f"feat{wi}"
                    )
                feat_ps = feat_tiles[wi]
                et = et_tiles.pop((wi, g))
                for j in range(GS):
                    kc = GS * g + j
                    nc.tensor.matmul(
                        feat_ps,
                        VT[:, kc * P : (kc + 1) * P],
                        et[:, j, :qwd],
                        start=(kc == 0),
                        stop=(kc == KC - 1),
                    )
                if g == NG - 1:
                    # den-final: contract the DVE partials with ones.
                    den_ps = denp.tile([P, qwd], f32, tag="den", name=f"den{wi}")
                    den_tiles[wi] = den_ps
                    np_ = len(partials[wi])
                    for pi, d in enumerate(partials[wi]):
                        for j in range(GS):
                            nc.tensor.matmul(
                                den_ps,
                                ones_sb,
                                d[:, j, :qwd],
                                start=(pi == 0 and j == 0),
                                stop=(pi == np_ - 1 and j == GS - 1),
                            )
                    fin_queue.append(wi)

            def emit_fin(wi):
                ws, qwd = W_SPANS[wi]
                feat_ps = feat_tiles.pop(wi)
                den_ps = den_tiles.pop(wi)
                partials[wi].clear()
                rec = small.tile([P, qwd], f32, tag="rec", name=f"rec{wi}")
                nc.vector.reciprocal(rec, den_ps)
                o_sb = op.tile([P, qwd], f32, tag="o", name=f"o{wi}")
                nc.vector.tensor_mul(o_sb, feat_ps, rec)
                nc.sync.dma_start(o.ap()[:, ws : ws + qwd], o_sb)

            fin_emitted = 0
            for idx in range(len(groups) + LAG + FLAG):
                if idx < len(groups):
                    emit_st(*groups[idx])
                if LAG <= idx < len(groups) + LAG:
                    emit_fd(*groups[idx - LAG])
                # finalize lags FLAG groups behind den-final emission so the
                # DVE reciprocal never stalls on in-flight PE den matmuls
                if fin_emitted < len(fin_queue) and (
                    idx >= len(groups) + LAG + FLAG - 1
                    or (fin_queue[fin_emitted] + 1) * NG + LAG + FLAG <= idx + 1
                ):
                    emit_fin(fin_queue[fin_emitted])
                    fin_emitted += 1
            while fin_emitted < len(fin_queue):
                emit_fin(fin_queue[fin_emitted])
                fin_emitted += 1

    nc.compile()
    return nc


def _get_runner():
    """Build the Bass module once and wrap it in a cached sharded jax callable.

    Mirrors concourse.bass2jax.run_bass_via_pjrt (the @via_axon execution
    path) but caches the jitted executable so repeated kernel() calls do not
    re-trace/re-compile.
    """
    if "runner" in _CACHE:
        return _CACHE["runner"]

    import jax
    from jax.experimental.shard_map import shard_map
    from jax.sharding import Mesh, PartitionSpec

    from concourse import bass2jax, mybir as _mybir

    bass2jax.install_neuronx_cc_hook()
    nc = _build()

    partition_name = nc.partition_id_tensor.name if nc.partition_id_tensor else None
    in_names, out_names, out_avals = [], [], []
    for alloc in nc.m.functions[0].allocations:
        if not isinstance(alloc, _mybir.MemoryLocationSet):
            continue
        name = alloc.memorylocations[0].name
        if alloc.kind == "ExternalInput":
            if name != partition_name:
                in_names.append(name)
        elif alloc.kind == "ExternalOutput":
            out_names.append(name)
            out_avals.append(
                jax.core.ShapedArray(
                    tuple(alloc.tensor_shape), _mybir.dt.np(alloc.dtype)
                )
            )
    n_params = len(in_names)
    all_in_names = in_names + out_names
    if partition_name is not None:
        all_in_names.append(partition_name)
    donate = tuple(range(n_params, n_params + len(out_names)))

    def _body(*args):
        operands = list(args)
        if partition_name is not None:
            operands.append(bass2jax.partition_id_tensor())
        outs = bass2jax._bass_exec_p.bind(
            *operands,
            out_avals=tuple(out_avals),
            in_names=tuple(all_in_names),
            out_names=tuple(out_names),
            lowering_input_output_aliases=(),
            sim_require_finite=True,
            sim_require_nnan=True,
            nc=nc,
        )
        return tuple(outs)

    devices = jax.devices()[:8]
    mesh = Mesh(np.asarray(devices), ("core",))
    in_specs = (PartitionSpec("core"),) * (n_params + len(out_names))
    out_specs = (PartitionSpec("core"),) * len(out_names)
    smapped = shard_map(
        _body, mesh=mesh, in_specs=in_specs, out_specs=out_specs, check_rep=False
    )
    sharded = jax.jit(smapped, donate_argnums=donate, keep_unused=True)

    out_shapes = [tuple(a.shape) for a in out_avals]
    out_dtypes = [a.dtype for a in out_avals]
    runner = {
        "fn": sharded,
        "smapped": smapped,
        "n_params": n_params,
        "in_names": in_names,
        "out_names": out_names,
        "out_shapes": out_shapes,
        "out_dtypes": out_dtypes,
        "nc": nc,
    }
    _CACHE["runner"] = runner
    return runner


def _run(in_maps):
    r = _get_runner()
    concat_in = [
        np.concatenate([np.asarray(m[name]) for m in in_maps], axis=0)
        for name in r["in_names"]
    ]
    concat_zeros = [
        np.zeros((8 * s[0], *s[1:]), d)
        for s, d in zip(r["out_shapes"], r["out_dtypes"])
    ]
    out_arrs = r["fn"](*concat_in, *concat_zeros)
    return [
        {
            name: np.asarray(out_arrs[i]).reshape(8, *r["out_shapes"][i])[c]
            for i, name in enumerate(r["out_names"])
        }
        for c in range(8)
    ]


def _make_in_maps(x, y, w_q, w_k, w_v):
    npbf = mybir.dt.np(bf16)
    x = np.ascontiguousarray(np.asarray(x, dtype=np.float32))
    y = np.ascontiguousarray(np.asarray(y, dtype=np.float32))
    bz, c, h, w = x.shape
    n = h * w
    xf = x.reshape(bz, c, n).astype(npbf)
    yf = y.reshape(bz, c, n).astype(npbf)
    wqT = np.asarray(w_q, dtype=np.float32).T  # [c, cq]
    wkT = np.asarray(w_k, dtype=np.float32).T
    z = np.zeros((c, 32 - CQ), np.float32)
    wq2 = np.concatenate([wqT, z, wqT, z, wqT, z, wqT], axis=1).astype(npbf)
    wk2 = np.concatenate([wkT, z, wkT, z, wkT, z, wkT], axis=1).astype(npbf)
    wvT = np.asarray(w_v, dtype=np.float32).T.astype(npbf)  # [c, c]
    in_maps = []
    for cid in range(8):
        b, qb = divmod(cid, 4)
        in_maps.append(
            {
                "y": np.ascontiguousarray(yf[b]),
                "yq": np.ascontiguousarray(yf[b][:, qb * NQ : (qb + 1) * NQ]),
                "x": np.ascontiguousarray(xf[b]),
                "wq": np.ascontiguousarray(wq2),
                "wk": np.ascontiguousarray(wk2),
                "wv": np.ascontiguousarray(wvT),
            }
        )
    return in_maps


def kernel(x, y, w_q, w_k, w_v):
    bz, c, h, w = np.asarray(x).shape
    n = h * w
    results = _run(_make_in_maps(x, y, w_q, w_k, w_v))
    feat = np.empty((bz, c, n), dtype=np.float32)
    for cid in range(8):
        b, qb = divmod(cid, 4)
        feat[b][:, qb * NQ : (qb + 1) * NQ] = results[cid]["o"]
    return feat.reshape(bz, c, h, w)
